# revision 17
# baseline (speedup 1.0000x reference)
"""Multi-head attention kernel for 8 TRN2 NeuronCores.

Problem: x(4,2048,1024) -> MHA(16 heads, d=64) -> out(4,2048,1024), f32.

Sharding: core c handles (batch b = c//2, seq half = c%2): it computes
attention outputs (incl. all projections) for its 1024 query rows over all 16
heads.  K/V projections for the full batch are computed locally per core (2x
redundant) which keeps cores fully independent - zero collectives.

Layouts: everything contracts over SBUF partitions.  Host pre-transposes x and
weights.  Scores are computed as ST[j,i] = K_h Q_h^T so the softmax exp runs
on ScalarE straight out of PSUM with the 1/8 scale fused, and rowsums come for
free from a ones-column interleaved into the staged V (M=128 matmul via a
negative-step 3D weight AP).  Normalization is a late reciprocal+broadcast
fused into the PSUM->SBUF evacuation.  V-projection bias is folded into the
output bias on host (bo_eff = bo + wo@bv).  Matmuls run in float32r (TF32-ish
rounding, ~1e-4 rel err, full PE rate).
"""

import numpy as np
from contextlib import ExitStack

P = 128
EMB = 1024
SEQ = 2048
QR = 1024          # query rows per core
NH = 16
HD = 64
EC = EMB // P      # 8 contraction chunks
RC = SEQ // P      # 16 seq row chunks
NB = 512           # free-dim block
SCALE = 0.125      # 1/sqrt(64)
VW = NH * 65       # staged V row width (ones col + 64 data cols per head)

_COMPILED = None   # (nc, names) cache


def _patch_tile_drain():
    """This walrus build only accepts ONE sync-wait per Drain instruction; the
    stock TileContext tail drain carries one wait per pending proc.  Split it
    into a chain of single-wait drains."""
    import concourse.tile as tile
    from concourse.vector_clock import ScopedClock, VectorClock

    if getattr(tile.TileContext, "_drain_patched", False):
        return

    def _drain_and_barrier(self, tick_clock, wait_clock):
        nc = self.nc
        gc = tick_clock.global_clock
        vals = eval(repr(gc).replace("VectorClock", ""))
        n = len(vals)
        for i, v in enumerate(vals):
            if v > 0:
                sub = VectorClock([vals[j] if j == i else 0 for j in range(n)])
                d = nc.sync.drain()
                wait_clock.add_sem_waits(d.ins, ScopedClock({None: sub}))
        nc.all_engine_barrier()
        popped = nc._tile_sem_poison_stack.pop()
        assert popped is self._sem_poison
        nc.clear_and_free_semaphores(list(self.sems.allocated().values()))
        nc.all_engine_barrier()

    tile.TileContext._drain_and_barrier = _drain_and_barrier
    tile.TileContext._drain_patched = True


def _build():
    import concourse.bass as bass
    import concourse.mybir as mybir
    import concourse.tile as tile

    _patch_tile_drain()

    f32 = mybir.dt.float32
    f32r = mybir.dt.float32r
    Exp = mybir.ActivationFunctionType.Exp

    nc = bass.Bass()

    # xt holds this core's batch transposed, with the core's 1024 query rows
    # FIRST (host pre-permutes; key/value row order is irrelevant to MHA).
    xt = nc.dram_tensor("xt", [EMB, SEQ], f32r, kind="ExternalInput")
    wqt = nc.dram_tensor("wqt", [EMB, EMB], f32r, kind="ExternalInput")
    wkt = nc.dram_tensor("wkt", [EMB, EMB], f32r, kind="ExternalInput")
    wvt = nc.dram_tensor("wvt", [EMB, EMB], f32r, kind="ExternalInput")
    wot = nc.dram_tensor("wot", [EMB, EMB], f32r, kind="ExternalInput")
    bqp = nc.dram_tensor("bqp", [P, EC], f32, kind="ExternalInput")
    bkp = nc.dram_tensor("bkp", [P, EC], f32, kind="ExternalInput")
    bob = nc.dram_tensor("bob", [P, EMB], f32, kind="ExternalInput")
    out = nc.dram_tensor("out", [QR, EMB], f32, kind="ExternalOutput")

    qstage = nc.dram_tensor("qstage", [EC, P, QR], f32r, kind="Internal")
    kstage = nc.dram_tensor("kstage", [EC, P, SEQ], f32r, kind="Internal")
    vstage = nc.dram_tensor("vstage", [RC, P, VW], f32r, kind="Internal")
    # softmax denominator bounce buffer (for partition-broadcast via DMA)
    bscr = nc.dram_tensor("bscr", [NH, 2, NB], f32, kind="Internal")

    with tile.TileContext(nc) as tc, ExitStack() as ctx:
        big = ctx.enter_context(tc.tile_pool(name="big", bufs=1))
        wpool = ctx.enter_context(tc.tile_pool(name="w", bufs=1))
        pspool = ctx.enter_context(tc.tile_pool(name="ps", bufs=2, space="PSUM"))
        stpool = ctx.enter_context(tc.tile_pool(name="st", bufs=2, space="PSUM"))
        otpool = ctx.enter_context(tc.tile_pool(name="ot", bufs=2, space="PSUM"))
        evac = ctx.enter_context(tc.tile_pool(name="evac", bufs=3))
        ptpool = ctx.enter_context(tc.tile_pool(name="pt", bufs=3))
        kpool = ctx.enter_context(tc.tile_pool(name="kp", bufs=2))
        qpool = ctx.enter_context(tc.tile_pool(name="qp", bufs=2))
        vpool = ctx.enter_context(tc.tile_pool(name="vp", bufs=2))
        nrm = ctx.enter_context(tc.tile_pool(name="nrm", bufs=2))
        misc = ctx.enter_context(tc.tile_pool(name="misc", bufs=1))

        # ---- persistent loads -------------------------------------------
        xt_sb = big.tile([P, EC * SEQ], f32r, tag="xt")
        for ec in range(EC):
            nc.sync.dma_start(xt_sb[:, ec * SEQ:(ec + 1) * SEQ],
                              xt[ec * P:(ec + 1) * P, :])
        bq_sb = misc.tile([P, EC], f32, tag="bq")
        nc.sync.dma_start(bq_sb[:], bqp[:])
        bk_sb = misc.tile([P, EC], f32, tag="bk")
        nc.sync.dma_start(bk_sb[:], bkp[:])
        bob_sb = misc.tile([P, EMB], f32, tag="bob")
        nc.sync.dma_start(bob_sb[:], bob[:])

        # ones columns of vstage (written once, before V tiles land)
        ones_sb = misc.tile([P, NH], f32, tag="ones")
        nc.vector.memset(ones_sb[:], 1.0)
        for rc in range(RC):
            ones_dst = bass.AP(vstage, rc * P * VW + HD, [[VW, P], [65, NH]])
            nc.sync.dma_start(ones_dst, ones_sb[:].bitcast(f32r))

        def load_w(which):
            w_sb = wpool.tile([P, EC * EMB], f32r, tag="w")
            for ec in range(EC):
                nc.sync.dma_start(w_sb[:, ec * EMB:(ec + 1) * EMB],
                                  which[ec * P:(ec + 1) * P, :])
            return w_sb

        # ---- Q projection: qstage[oc][p, i] = (x_q @ wq.T + bq).T -------
        w_sb = load_w(wqt)
        for oc in range(EC):
            for ib in range(QR // NB):
                ps = pspool.tile([P, NB], f32, tag="ps")
                for ec in range(EC):
                    nc.tensor.matmul(
                        ps[:],
                        w_sb[:, ec * EMB + oc * P: ec * EMB + (oc + 1) * P],
                        xt_sb[:, ec * SEQ + ib * NB: ec * SEQ + (ib + 1) * NB],
                        start=(ec == 0), stop=(ec == EC - 1))
                ev = evac.tile([P, NB], f32r, tag="ev")
                nc.vector.tensor_scalar_add(ev[:], ps[:], bq_sb[:, oc:oc + 1])
                nc.sync.dma_start(qstage[oc, :, ib * NB:(ib + 1) * NB], ev[:])

        # ---- V projection (before K so attention can overlap K) ---------
        w_sb = load_w(wvt)
        for rc in range(RC):
            for ob in range(2):
                ps = pspool.tile([P, NB], f32, tag="ps")
                for ec in range(EC):
                    nc.tensor.matmul(
                        ps[:],
                        xt_sb[:, ec * SEQ + rc * P: ec * SEQ + (rc + 1) * P],
                        w_sb[:, ec * EMB + ob * NB: ec * EMB + (ob + 1) * NB],
                        start=(ec == 0), stop=(ec == EC - 1))
                ev = evac.tile([P, NB], f32r, tag="ev")
                nc.vector.tensor_copy(ev[:], ps[:])
                vdst = bass.AP(vstage, rc * P * VW + (ob * 8) * 65,
                               [[VW, P], [65, 8], [1, HD]])
                nc.sync.dma_start(
                    vdst, ev[:].rearrange("p (a b) -> p a b", a=8))

        # ---- K projection ------------------------------------------------
        w_sb = load_w(wkt)
        for oc in range(EC):
            for jb in range(SEQ // NB):
                ps = pspool.tile([P, NB], f32, tag="ps")
                for ec in range(EC):
                    nc.tensor.matmul(
                        ps[:],
                        w_sb[:, ec * EMB + oc * P: ec * EMB + (oc + 1) * P],
                        xt_sb[:, ec * SEQ + jb * NB: ec * SEQ + (jb + 1) * NB],
                        start=(ec == 0), stop=(ec == EC - 1))
                ev = evac.tile([P, NB], f32r, tag="ev")
                nc.vector.tensor_scalar_add(ev[:], ps[:], bk_sb[:, oc:oc + 1])
                nc.sync.dma_start(kstage[oc, :, jb * NB:(jb + 1) * NB], ev[:])

        # ---- attention: att_sb[c-chunk t partitions, r] ------------------
        # shares the xt slot (xt's last read was the K projection)
        att_sb = big.tile([P, EC * QR], f32r, tag="xt")
        for t in range(NH // 2):
            ktp = kpool.tile([P, SEQ], f32r, tag="kt")
            nc.sync.dma_start(ktp[:], kstage[t, :, :])
            qtp = qpool.tile([P, QR], f32r, tag="qt")
            nc.sync.dma_start(qtp[:], qstage[t, :, :])
            for e in range(2):
                h = 2 * t + e
                vh = vpool.tile([P, RC * 65], f32r, tag="vh")
                vsrc = bass.AP(vstage, h * 65,
                               [[VW, P], [P * VW, RC], [1, 65]])
                nc.sync.dma_start(
                    vh[:].rearrange("p (a b) -> p a b", a=RC), vsrc)
                for ib in range(QR // NB):
                    ot_ps = otpool.tile([P, NB], f32, tag="ot")
                    for jc2 in range(RC // 2):
                        st_ps = stpool.tile([P, 2 * NB], f32, tag="st")
                        for u in range(2):
                            jc = jc2 * 2 + u
                            nc.tensor.matmul(
                                st_ps[:, u * NB:(u + 1) * NB],
                                ktp[e * HD:(e + 1) * HD,
                                    jc * P:(jc + 1) * P],
                                qtp[e * HD:(e + 1) * HD,
                                    ib * NB:(ib + 1) * NB],
                                start=True, stop=True)
                        pt = ptpool.tile([P, 2 * NB], f32r, tag="pt")
                        nc.scalar.activation(pt[:], st_ps[:], Exp, scale=SCALE)
                        for u in range(2):
                            jc = jc2 * 2 + u
                            nc.tensor.matmul(
                                ot_ps[0:65, :],
                                vh[:, jc * 65:(jc + 1) * 65],
                                pt[:, u * NB:(u + 1) * NB],
                                start=(jc == 0), stop=(jc == RC - 1))
                    # rows 0..63 = head output^T, row 64 = softmax denominator.
                    # Broadcast the denominator row across 64 partitions via a
                    # DRAM bounce (no gpsimd ucode in this toolchain), then a
                    # single fused divide evacuates + normalizes.
                    rs = nrm.tile([P, NB], f32, tag="rs")
                    nc.vector.reciprocal(rs[64:65, :], ot_ps[64:65, :])
                    nc.sync.dma_start(bscr[h, ib, :], rs[64:65, :])
                    bc = nrm.tile([P, NB], f32, tag="bc")
                    bsrc = bass.AP(bscr, (h * 2 + ib) * NB, [[0, HD], [1, NB]])
                    nc.sync.dma_start(bc[0:HD, :], bsrc)
                    nc.vector.tensor_mul(
                        att_sb[e * HD:(e + 1) * HD,
                               t * QR + ib * NB: t * QR + (ib + 1) * NB],
                        ot_ps[0:HD, :], bc[0:HD, :])

        # ---- output projection ------------------------------------------
        w_sb = load_w(wot)
        for rc8 in range(QR // P):
            for ob in range(2):
                ps = pspool.tile([P, NB], f32, tag="ps")
                for cc in range(EC):
                    nc.tensor.matmul(
                        ps[:],
                        att_sb[:, cc * QR + rc8 * P: cc * QR + (rc8 + 1) * P],
                        w_sb[:, cc * EMB + ob * NB: cc * EMB + (ob + 1) * NB],
                        start=(cc == 0), stop=(cc == EC - 1))
                ev = evac.tile([P, NB], f32, tag="evo")
                nc.vector.tensor_add(ev[:], ps[:], bob_sb[:, ob * NB:(ob + 1) * NB])
                nc.sync.dma_start(
                    out[rc8 * P:(rc8 + 1) * P, ob * NB:(ob + 1) * NB], ev[:])

    # gpsimd PartitionBroadcast needs a ucode library selected; mirror
    # Bacc.insert_library_loads on this plain Bass module.
    import bass_rust as _bass_rust
    from concourse.library_config import all_libraries, standard

    inst_type_to_lib_mask = {}
    for lib in all_libraries:
        for inst_type in lib.instructions:
            inst_type_to_lib_mask[inst_type] = inst_type_to_lib_mask.get(
                inst_type, 0) | (1 << lib.index)
    _bass_rust.insert_library_loads(
        nc, inst_type_to_lib_mask, len(all_libraries), standard.index)

    _split_multi_waits(nc, mybir)

    return nc


def _split_multi_waits(nc, mybir):
    """This walrus build accepts at most ONE sync-wait per instruction; Tile
    emits several.  Hoist all but the last wait onto single-wait NoOps placed
    immediately before the instruction on the same engine."""
    nop_id = [0]
    for fn in nc.m.functions:
        for bb in fn.blocks:
            out = []
            for inst in bb.instructions:
                si = inst.sync_info
                if si is not None and si.on_wait is not None \
                        and len(si.on_wait) > 1:
                    waits = list(si.on_wait)
                    for w in waits[:-1]:
                        nop = mybir.InstNoOp(
                            name=f"I-waitsplit-{nop_id[0]}", ins=[], outs=[])
                        nop_id[0] += 1
                        nop.engine = inst.engine
                        nop.sync_info = mybir.SyncInfo(
                            on_wait=[w], on_update=[])
                        out.append(nop)
                    inst.sync_info = mybir.SyncInfo(
                        on_wait=[waits[-1]],
                        on_update=list(si.on_update or []))
                out.append(inst)
            bb.instructions = out


def _get_compiled():
    global _COMPILED
    if _COMPILED is None:
        _COMPILED = _build()
    return _COMPILED


def kernel(x, wq, bq, wk, bk, wv, bv, wo, bo, _want_results_obj=False,
           **run_kwargs):
    from concourse.bass_utils import run_bass_kernel_spmd

    x = np.asarray(x, dtype=np.float32)
    wq = np.asarray(wq, dtype=np.float32)
    bq = np.asarray(bq, dtype=np.float32)
    wk = np.asarray(wk, dtype=np.float32)
    bk = np.asarray(bk, dtype=np.float32)
    wv = np.asarray(wv, dtype=np.float32)
    bv = np.asarray(bv, dtype=np.float32)
    wo = np.asarray(wo, dtype=np.float32)
    bo = np.asarray(bo, dtype=np.float32)

    bs, seq, emb = x.shape
    assert (bs, seq, emb) == (4, SEQ, EMB)

    nc = _get_compiled()

    shared = {
        "wqt": np.ascontiguousarray(wq.T),
        "wkt": np.ascontiguousarray(wk.T),
        "wvt": np.ascontiguousarray(wv.T),
        "wot": np.ascontiguousarray(wo.T),
        "bqp": np.ascontiguousarray(bq.reshape(EC, P).T),
        "bkp": np.ascontiguousarray(bk.reshape(EC, P).T),
        "bob": np.ascontiguousarray(
            np.broadcast_to(bo + wo @ bv, (P, EMB))),
    }
    in_maps = []
    for c in range(8):
        b, hf = c // 2, c % 2
        xb = x[b]
        # this core's query rows first; row order of keys/values is irrelevant
        xb_perm = np.concatenate(
            [xb[hf * QR:(hf + 1) * QR], xb[(1 - hf) * QR:(2 - hf) * QR]], axis=0)
        in_maps.append({
            "xt": np.ascontiguousarray(xb_perm.T),
            **shared,
        })

    res = run_bass_kernel_spmd(nc, in_maps, core_ids=list(range(8)),
                               **run_kwargs)

    outp = np.empty((bs, seq, emb), dtype=np.float32)
    for c in range(8):
        b, hf = c // 2, c % 2
        outp[b, hf * QR:(hf + 1) * QR, :] = res.results[c]["out"]
    if _want_results_obj:
        return outp, res
    return outp


# revision 34
# speedup vs baseline: 930.0683x; 930.0683x over previous
"""Multi-head attention kernel for 8 TRN2 NeuronCores.

Problem: x(4,2048,1024) -> MHA(16 heads, d=64) -> out(4,2048,1024), f32.

Sharding: core c handles (batch b = c//2, seq half = c%2): it computes
attention outputs (incl. all projections) for its 1024 query rows over all 16
heads.  K/V projections for the full batch are computed locally per core (2x
redundant) which keeps cores fully independent - zero collectives.

Layouts: everything contracts over SBUF partitions.  Host pre-transposes x and
weights.  Scores are computed as ST[j,i] = K_h Q_h^T so the softmax exp runs
on ScalarE straight out of PSUM with the 1/8 scale fused; softmax denominators
come for free from a ones-column appended to each V tile (M=65 PV matmul, row
64 = rowsum).  Normalization is reciprocal + a DRAM-bounce partition-broadcast
multiplied during the PSUM->SBUF evacuation.  V-projection bias is folded into
the output bias on host (bo_eff = bo + wo@bv).  Matmuls run in float32r
(TF32-ish rounding, ~1.6e-4 rel err end to end, full PE rate at N>=256).

Schedule: Q projection -> V projection (wv prefetched into the att slot, wot
into the wq slot) -> per head pair: K projection (SBUF-resident, fills PE
while softmax keeps ScalarE busy) + attention -> output projection (first
row-half starts during the last pair).  Staging writes ride the gpsimd SWDGE
queues so they never queue behind HWDGE prefetch reads.  This walrus build
accepts only ONE sync-wait per instruction, so a post-pass splits multi-wait
instructions into single-wait NoOps (_split_multi_waits), and custom DVE ops /
gpsimd ucode are unavailable (hence the DMA-based broadcast).
"""

import numpy as np
from contextlib import ExitStack

P = 128
EMB = 1024
SEQ = 2048
QR = 1024          # query rows per core
NH = 16
HD = 64
EC = EMB // P      # 8 contraction chunks
RC = SEQ // P      # 16 seq row chunks
NB = 512           # free-dim block
SCALE = 0.125      # 1/sqrt(64)

_COMPILED = None   # (nc, names) cache


def _patch_tile_drain():
    """This walrus build only accepts ONE sync-wait per Drain instruction; the
    stock TileContext tail drain carries one wait per pending proc.  Split it
    into a chain of single-wait drains."""
    import concourse.tile as tile
    from concourse.vector_clock import ScopedClock, VectorClock

    if getattr(tile.TileContext, "_drain_patched", False):
        return

    def _drain_and_barrier(self, tick_clock, wait_clock):
        nc = self.nc
        gc = tick_clock.global_clock
        vals = eval(repr(gc).replace("VectorClock", ""))
        n = len(vals)
        for i, v in enumerate(vals):
            if v > 0:
                sub = VectorClock([vals[j] if j == i else 0 for j in range(n)])
                d = nc.sync.drain()
                wait_clock.add_sem_waits(d.ins, ScopedClock({None: sub}))
        nc.all_engine_barrier()
        popped = nc._tile_sem_poison_stack.pop()
        assert popped is self._sem_poison
        nc.clear_and_free_semaphores(list(self.sems.allocated().values()))
        nc.all_engine_barrier()

    tile.TileContext._drain_and_barrier = _drain_and_barrier
    tile.TileContext._drain_patched = True


def _build():
    import concourse.bass as bass
    import concourse.mybir as mybir
    import concourse.tile as tile

    _patch_tile_drain()

    f32 = mybir.dt.float32
    f32r = mybir.dt.float32r
    Exp = mybir.ActivationFunctionType.Exp

    nc = bass.Bass()

    # xt holds this core's batch transposed, with the core's 1024 query rows
    # FIRST (host pre-permutes; key/value row order is irrelevant to MHA).
    xt = nc.dram_tensor("xt", [EMB, SEQ], f32r, kind="ExternalInput")
    wqt = nc.dram_tensor("wqt", [EMB, EMB], f32r, kind="ExternalInput")
    wkt = nc.dram_tensor("wkt", [EMB, EMB], f32r, kind="ExternalInput")
    wvt = nc.dram_tensor("wvt", [EMB, EMB], f32r, kind="ExternalInput")
    wot = nc.dram_tensor("wot", [EMB, EMB], f32r, kind="ExternalInput")
    bqp = nc.dram_tensor("bqp", [P, EC], f32, kind="ExternalInput")
    bkp = nc.dram_tensor("bkp", [P, EC], f32, kind="ExternalInput")
    bob = nc.dram_tensor("bob", [P, EMB], f32, kind="ExternalInput")
    out = nc.dram_tensor("out", [QR, EMB], f32, kind="ExternalOutput")

    qstage = nc.dram_tensor("qstage", [EC, P, QR], f32r, kind="Internal")
    vstage = nc.dram_tensor("vstage", [RC, P, EMB], f32r, kind="Internal")
    # softmax denominator bounce buffer (for partition-broadcast via DMA)
    bscr = nc.dram_tensor("bscr", [NH, 2, NB], f32, kind="Internal")

    with tile.TileContext(nc) as tc, ExitStack() as ctx:
        big = ctx.enter_context(tc.tile_pool(name="big", bufs=1))
        wpool = ctx.enter_context(tc.tile_pool(name="w", bufs=1))
        pspool = ctx.enter_context(tc.tile_pool(name="ps", bufs=2, space="PSUM"))
        stpool = ctx.enter_context(tc.tile_pool(name="st", bufs=2, space="PSUM"))
        otpool = ctx.enter_context(tc.tile_pool(name="ot", bufs=2, space="PSUM"))
        evac = ctx.enter_context(tc.tile_pool(name="evac", bufs=3))
        ptpool = ctx.enter_context(tc.tile_pool(name="pt", bufs=3))
        kpool = ctx.enter_context(tc.tile_pool(name="kp", bufs=2))
        wkpool = ctx.enter_context(tc.tile_pool(name="wk", bufs=2))
        qpool = ctx.enter_context(tc.tile_pool(name="qp", bufs=2))
        vpool = ctx.enter_context(tc.tile_pool(name="vp", bufs=2))
        nrm = ctx.enter_context(tc.tile_pool(name="nrm", bufs=2))
        misc = ctx.enter_context(tc.tile_pool(name="misc", bufs=1))

        # ---- persistent loads -------------------------------------------
        # DMA queue order tracks emission order, so the Q-projection's
        # critical inputs (wq, then x's query columns) are emitted first.
        def load_w(which, pool, tag):
            w_sb = pool.tile([P, EC * EMB], f32r, tag=tag, name="w_sb")
            for ec in range(EC):
                nc.sync.dma_start(w_sb[:, ec * EMB:(ec + 1) * EMB],
                                  which[ec * P:(ec + 1) * P, :])
            return w_sb

        bq_sb = misc.tile([P, EC], f32, tag="bq")
        nc.sync.dma_start(bq_sb[:], bqp[:])
        bk_sb = misc.tile([P, EC], f32, tag="bk")
        nc.sync.dma_start(bk_sb[:], bkp[:])
        bob_sb = misc.tile([P, EMB], f32, tag="bob")
        nc.sync.dma_start(bob_sb[:], bob[:])
        wq_sb = load_w(wqt, wpool, "w")
        xt_sb = big.tile([P, EC * SEQ], f32r, tag="xt")
        for ec in range(EC):
            nc.sync.dma_start(xt_sb[:, ec * SEQ: ec * SEQ + QR],
                              xt[ec * P:(ec + 1) * P, 0:QR])
        # wv shares the att slot (disjoint lifetimes); interleave its halves
        # with the x key-column halves in the order the V projection consumes
        wv_sb = big.tile([P, EC * EMB], f32r, tag="att", name="wv_sb")
        for half in range(2):
            for ec in range(EC):
                nc.sync.dma_start(
                    wv_sb[:, ec * EMB + half * NB: ec * EMB + (half + 1) * NB],
                    wvt[ec * P:(ec + 1) * P, half * NB:(half + 1) * NB])
            for ec in range(EC):
                nc.sync.dma_start(
                    xt_sb[:, ec * SEQ + QR + half * NB: ec * SEQ + QR + (half + 1) * NB],
                    xt[ec * P:(ec + 1) * P, QR + half * NB: QR + (half + 1) * NB])
        # ---- Q projection: qstage[oc][p, i] = (x_q @ wq.T + bq).T -------
        for oc in range(EC):
            for ib in range(QR // NB):
                ps = pspool.tile([P, NB], f32, tag="ps")
                for ec in range(EC):
                    nc.tensor.matmul(
                        ps[:],
                        wq_sb[:, ec * EMB + oc * P: ec * EMB + (oc + 1) * P],
                        xt_sb[:, ec * SEQ + ib * NB: ec * SEQ + (ib + 1) * NB],
                        start=(ec == 0), stop=(ec == EC - 1))
                ev = evac.tile([P, NB], f32r, tag="ev")
                nc.vector.tensor_scalar_add(ev[:], ps[:], bq_sb[:, oc:oc + 1])
                nc.gpsimd.dma_start(qstage[oc, :, ib * NB:(ib + 1) * NB], ev[:])

        # wot takes the wq slot; its DMAs overlap the V projection
        wot_sb = load_w(wot, wpool, "w")

        def load_wk(t):
            wk_t = wkpool.tile([P, EC * P], f32r, tag="wk", name="wk_t")
            for ec in range(EC):
                nc.sync.dma_start(
                    wk_t[:, ec * P:(ec + 1) * P],
                    wkt[ec * P:(ec + 1) * P, t * P:(t + 1) * P])
            return wk_t

        wk_next = load_wk(0)

        # ---- V projection (before K so attention can overlap K) ---------
        for ob in range(2):
            for rc in range(RC):
                ps = pspool.tile([P, NB], f32, tag="ps")
                for ec in range(EC):
                    nc.tensor.matmul(
                        ps[:],
                        xt_sb[:, ec * SEQ + rc * P: ec * SEQ + (rc + 1) * P],
                        wv_sb[:, ec * EMB + ob * NB: ec * EMB + (ob + 1) * NB],
                        start=(ec == 0), stop=(ec == EC - 1))
                ev = evac.tile([P, NB], f32r, tag="ev")
                nc.vector.tensor_copy(ev[:], ps[:])
                nc.gpsimd.dma_start(vstage[rc, :, ob * NB:(ob + 1) * NB], ev[:])

        def out_proj(rc8s):
            for rc8 in rc8s:
                for ob in range(2):
                    ps = pspool.tile([P, NB], f32, tag="ps", name="ps")
                    for cc in range(EC):
                        nc.tensor.matmul(
                            ps[:],
                            att_sb[:, cc * QR + rc8 * P: cc * QR + (rc8 + 1) * P],
                            wot_sb[:, cc * EMB + ob * NB: cc * EMB + (ob + 1) * NB],
                            start=(cc == 0), stop=(cc == EC - 1))
                    ev = evac.tile([P, NB], f32, tag="evo", name="ev")
                    nc.vector.tensor_add(
                        ev[:], ps[:], bob_sb[:, ob * NB:(ob + 1) * NB])
                    nc.gpsimd.dma_start(
                        out[rc8 * P:(rc8 + 1) * P, ob * NB:(ob + 1) * NB], ev[:])

        # ---- K projection interleaved with attention ---------------------
        # K o-chunk t == head pair t; it stays in SBUF and feeds attention
        # directly, and the next pair's K matmuls give PE filler work while
        # the current pair's softmax keeps ScalarE busy.  wk is loaded as thin
        # per-pair column slices (the big weight slot is occupied by wot).
        att_sb = big.tile([P, EC * QR], f32r, tag="att")
        for t in range(NH // 2):
            wk_t = wk_next
            if t + 1 < NH // 2:
                wk_next = load_wk(t + 1)
            ktp = kpool.tile([P, SEQ], f32r, tag="kt")
            for jb in range(SEQ // NB):
                ps = pspool.tile([P, NB], f32, tag="ps")
                for ec in range(EC):
                    nc.tensor.matmul(
                        ps[:],
                        wk_t[:, ec * P:(ec + 1) * P],
                        xt_sb[:, ec * SEQ + jb * NB: ec * SEQ + (jb + 1) * NB],
                        start=(ec == 0), stop=(ec == EC - 1))
                nc.vector.tensor_scalar_add(
                    ktp[:, jb * NB:(jb + 1) * NB], ps[:], bk_sb[:, t:t + 1])
            qtp = qpool.tile([P, QR], f32r, tag="qt")
            nc.sync.dma_start(qtp[:], qstage[t, :, :])
            # last pair runs ib-major so half the output projection can start
            # while its second i-block is still in softmax
            ebs = ([(e, ib) for e in range(2) for ib in range(QR // NB)]
                   if t < NH // 2 - 1 else
                   [(e, ib) for ib in range(QR // NB) for e in range(2)])
            vh_tiles = {}
            for e, ib in ebs:
                h = 2 * t + e
                if e not in vh_tiles:
                    vh = vpool.tile([P, RC * 65], f32r, tag="vh", name="vh")
                    vdst = vh[:].rearrange("p (a b) -> p a b", a=RC)
                    vsrc = bass.AP(vstage, h * HD,
                                   [[EMB, P], [P * EMB, RC], [1, HD]])
                    nc.sync.dma_start(vdst[:, :, 0:HD], vsrc)
                    ones_ap = bass.AP(vh.tensor, vh.offset + HD,
                                      [list(vh.ap[0]), [65, RC]])
                    nc.vector.memset(ones_ap.bitcast(f32), 1.0)
                    vh_tiles[e] = vh
                vh = vh_tiles[e]
                if True:
                    ot_ps = otpool.tile([P, NB], f32, tag="ot")
                    for jc2 in range(RC // 2):
                        st_ps = stpool.tile([P, 2 * NB], f32, tag="st")
                        for u in range(2):
                            jc = jc2 * 2 + u
                            nc.tensor.matmul(
                                st_ps[:, u * NB:(u + 1) * NB],
                                ktp[e * HD:(e + 1) * HD,
                                    jc * P:(jc + 1) * P],
                                qtp[e * HD:(e + 1) * HD,
                                    ib * NB:(ib + 1) * NB],
                                start=True, stop=True)
                        pt = ptpool.tile([P, 2 * NB], f32r, tag="pt")
                        nc.scalar.activation(pt[:], st_ps[:], Exp, scale=SCALE)
                        for u in range(2):
                            jc = jc2 * 2 + u
                            nc.tensor.matmul(
                                ot_ps[0:65, :],
                                vh[:, jc * 65:(jc + 1) * 65],
                                pt[:, u * NB:(u + 1) * NB],
                                start=(jc == 0), stop=(jc == RC - 1))
                    # rows 0..63 = head output^T, row 64 = softmax denominator.
                    # Broadcast the denominator row across 64 partitions via a
                    # DRAM bounce (no gpsimd ucode in this toolchain), then a
                    # single fused divide evacuates + normalizes.
                    rs = nrm.tile([P, NB], f32, tag="rs")
                    nc.vector.reciprocal(rs[64:65, :], ot_ps[64:65, :])
                    nc.gpsimd.dma_start(bscr[h, ib, :], rs[64:65, :])
                    bc = nrm.tile([P, NB], f32, tag="bc")
                    bsrc = bass.AP(bscr, (h * 2 + ib) * NB, [[0, HD], [1, NB]])
                    nc.gpsimd.dma_start(bc[0:HD, :], bsrc)
                    nc.vector.tensor_mul(
                        att_sb[e * HD:(e + 1) * HD,
                               t * QR + ib * NB: t * QR + (ib + 1) * NB],
                        ot_ps[0:HD, :], bc[0:HD, :])
                if t == NH // 2 - 1 and e == 1:
                    out_proj(range(ib * (QR // P // 2), (ib + 1) * (QR // P // 2)))

        # ---- output projection (emitted inside out_proj) -----------------

    # gpsimd PartitionBroadcast needs a ucode library selected; mirror
    # Bacc.insert_library_loads on this plain Bass module.
    import bass_rust as _bass_rust
    from concourse.library_config import all_libraries, standard

    inst_type_to_lib_mask = {}
    for lib in all_libraries:
        for inst_type in lib.instructions:
            inst_type_to_lib_mask[inst_type] = inst_type_to_lib_mask.get(
                inst_type, 0) | (1 << lib.index)
    _bass_rust.insert_library_loads(
        nc, inst_type_to_lib_mask, len(all_libraries), standard.index)

    _split_multi_waits(nc, mybir)

    return nc


def _split_multi_waits(nc, mybir):
    """This walrus build accepts at most ONE sync-wait per instruction; Tile
    emits several.  Hoist all but the last wait onto single-wait NoOps placed
    immediately before the instruction on the same engine."""
    nop_id = [0]
    for fn in nc.m.functions:
        for bb in fn.blocks:
            out = []
            for inst in bb.instructions:
                si = inst.sync_info
                if si is not None and si.on_wait is not None \
                        and len(si.on_wait) > 1:
                    waits = list(si.on_wait)
                    for w in waits[:-1]:
                        nop = mybir.InstNoOp(
                            name=f"I-waitsplit-{nop_id[0]}", ins=[], outs=[])
                        nop_id[0] += 1
                        nop.engine = inst.engine
                        nop.sync_info = mybir.SyncInfo(
                            on_wait=[w], on_update=[])
                        out.append(nop)
                    inst.sync_info = mybir.SyncInfo(
                        on_wait=[waits[-1]],
                        on_update=list(si.on_update or []))
                out.append(inst)
            bb.instructions = out


def _get_compiled():
    global _COMPILED
    if _COMPILED is None:
        _COMPILED = _build()
    return _COMPILED


def kernel(x, wq, bq, wk, bk, wv, bv, wo, bo, _want_results_obj=False,
           **run_kwargs):
    from concourse.bass_utils import run_bass_kernel_spmd

    x = np.asarray(x, dtype=np.float32)
    wq = np.asarray(wq, dtype=np.float32)
    bq = np.asarray(bq, dtype=np.float32)
    wk = np.asarray(wk, dtype=np.float32)
    bk = np.asarray(bk, dtype=np.float32)
    wv = np.asarray(wv, dtype=np.float32)
    bv = np.asarray(bv, dtype=np.float32)
    wo = np.asarray(wo, dtype=np.float32)
    bo = np.asarray(bo, dtype=np.float32)

    bs, seq, emb = x.shape
    assert (bs, seq, emb) == (4, SEQ, EMB)

    nc = _get_compiled()

    shared = {
        "wqt": np.ascontiguousarray(wq.T),
        "wkt": np.ascontiguousarray(wk.T),
        "wvt": np.ascontiguousarray(wv.T),
        "wot": np.ascontiguousarray(wo.T),
        "bqp": np.ascontiguousarray(bq.reshape(EC, P).T),
        "bkp": np.ascontiguousarray(bk.reshape(EC, P).T),
        "bob": np.ascontiguousarray(
            np.broadcast_to(bo + wo @ bv, (P, EMB))),
    }
    in_maps = []
    for c in range(8):
        b, hf = c // 2, c % 2
        xb = x[b]
        # this core's query rows first; row order of keys/values is irrelevant
        xb_perm = np.concatenate(
            [xb[hf * QR:(hf + 1) * QR], xb[(1 - hf) * QR:(2 - hf) * QR]], axis=0)
        in_maps.append({
            "xt": np.ascontiguousarray(xb_perm.T),
            **shared,
        })

    res = run_bass_kernel_spmd(nc, in_maps, core_ids=list(range(8)),
                               **run_kwargs)

    outp = np.empty((bs, seq, emb), dtype=np.float32)
    for c in range(8):
        b, hf = c // 2, c % 2
        outp[b, hf * QR:(hf + 1) * QR, :] = res.results[c]["out"]
    if _want_results_obj:
        return outp, res
    return outp
